# revision 1
# baseline (speedup 1.0000x reference)
"""Bidirectional attention (RoPE-variant) Trainium2 kernel.

Reference computation (B=4, T=2048, C=2048, H=16, D=128):
    q = (x @ wq.T) -> rotary; k = (x @ wk.T) -> rotary; v = x @ wv.T
    y = softmax(q k^T / sqrt(D)) v ; out = y @ wo.T

Sharding over 8 NeuronCores: core c -> (batch b = c//2, head-group g = c%2).
Each core computes q/k/v projections for its batch restricted to its 8 heads,
full attention for those heads, and a partial o-projection (contracting its
1024 hidden columns).  The host sums the two partial outputs per batch — no
device collectives, and every core does exactly 1/8 of the matmul FLOPs.

On-device layout is "transposed" end-to-end so the PE (out = lhsT.T @ rhs)
never needs an explicit transpose.  V is computed first (natural (t, d)
layout) and spilled to DRAM; Q^T/K^T are produced head-by-head (rotary
applied straight out of PSUM) and spilled as bf16 — K optionally as a
bf16 hi+lo pair (SPLIT_K) whose two accumulating matmuls remove the K-side
quantization error from the scores.  Attention for head h-1 is emitted
between the Q/K blocks of head h so its exp() work (ScalarE) and tree-sums
(VectorE, gpsimd partition_all_reduce, reciprocal_approx_fast) hide under
projection matmuls instead of forming an ACT-bound serial phase.  The
partial o-projection is interleaved into the last head's attention loop.
"""

import sys

if "/opt/trn_rl_repo" not in sys.path:
    sys.path.insert(0, "/opt/trn_rl_repo")

import os
import numpy as np
import ml_dtypes

B, T, C = 4, 2048, 2048
H_TOT = 16
D = 128
HG = 8            # heads per core
JG = HG * D       # 1024 hidden columns per head-group
N_CORES = 8
CT = C // 128     # 16 c-tiles (contraction over channels)
TT = T // 128     # 16 t-tiles
QCH = T // 512    # 4 query chunks of 512
KT = T // 128     # 16 key tiles of 128
SCALE = 1.0 / float(np.sqrt(D))

BF16 = ml_dtypes.bfloat16
SPLIT_K = os.environ.get("SPLIT_K", "1") == "1"

_CACHE = {}


def _build_bass():
    import concourse.tile as tile
    import concourse.bass_isa as bass_isa
    from concourse import bacc, mybir
    from concourse.bass import ts
    from contextlib import ExitStack

    bf16 = mybir.dt.bfloat16
    f32 = mybir.dt.float32

    nc = bacc.Bacc("TRN2", target_bir_lowering=False, debug=False)

    x_pack = nc.dram_tensor("x_pack", [128, CT, T], bf16, kind="ExternalInput")
    wq_pack = nc.dram_tensor("wq_pack", [HG, 128, CT, 128], bf16, kind="ExternalInput")
    wk_pack = nc.dram_tensor("wk_pack", [HG, 128, CT, 128], bf16, kind="ExternalInput")
    wv_pack = nc.dram_tensor("wv_pack", [128, CT, JG], bf16, kind="ExternalInput")
    wo_pack = nc.dram_tensor("wo_pack", [128, HG, C], bf16, kind="ExternalInput")
    # cs_pack rows 0:64 = cos^T, rows 64:128 = sin^T
    cs_pack = nc.dram_tensor("cs_pack", [128, T], f32, kind="ExternalInput")
    out = nc.dram_tensor("out", [T, C], f32, kind="ExternalOutput")

    nk = 2 if SPLIT_K else 1

    with tile.TileContext(nc) as tc, ExitStack() as ctx:
        # Pools opened in lifetime order: persistent + attention scratch first
        # (bottom of the SBUF stack), then phase-1 pools on top, so attention
        # tiles never alias phase-1 space (aliasing would add release deps
        # that serialize the phases).
        persist = ctx.enter_context(tc.tile_pool(name="persist", bufs=1))
        p2k = ctx.enter_context(tc.tile_pool(name="p2k", bufs=1))
        p2q = ctx.enter_context(tc.tile_pool(name="p2q", bufs=2))
        p2u = ctx.enter_context(tc.tile_pool(name="p2u", bufs=2))
        p2sm = ctx.enter_context(tc.tile_pool(name="p2sm", bufs=1))
        p2v = ctx.enter_context(tc.tile_pool(name="p2v", bufs=1))
        dram = ctx.enter_context(tc.tile_pool(name="dram", bufs=1, space="DRAM"))
        ps_sc = ctx.enter_context(tc.tile_pool(name="ps_sc", bufs=2, space="PSUM"))
        ps_gen = ctx.enter_context(tc.tile_pool(name="ps_gen", bufs=4, space="PSUM"))

        yt_sb = persist.tile([128, HG, T], bf16)      # y^T, (d, h, t)

        qt_dram = [
            dram.tile([128, T], bf16, tag=f"qt{h}", name=f"qt_dram{h}")
            for h in range(HG)
        ]
        kt_dram = [
            dram.tile([128, nk, T], bf16, tag=f"kt{h}", name=f"kt_dram{h}")
            for h in range(HG)
        ]
        v_dram = dram.tile([128, TT, JG], bf16, name="v_dram")

        def oproj_group(g):
            # o-proj tile-group g needs q-chunk g of ALL heads, complete at
            # (h = HG-1, qc = g); emitted there so it hides under attention.
            for tm in range(4 * g, 4 * g + 4):
                stg = p3stg.tile([128, C], f32, tag="ostg", name=f"ostg{tm}")
                pss = [
                    ps_gen.tile([128, 512], f32, tag="ps", name=f"pso{tm}_{c}")
                    for c in range(C // 512)
                ]
                for ji in range(HG):
                    for cch in range(C // 512):
                        nc.tensor.matmul(
                            pss[cch][:],
                            lhsT=yt_sb[:, ji, ts(tm, 128)],
                            rhs=wo_sb[:, ji, ts(cch, 512)],
                            start=(ji == 0),
                            stop=(ji == HG - 1),
                        )
                for cch in range(C // 512):
                    nc.vector.tensor_copy(
                        out=stg[:, ts(cch, 512)], in_=pss[cch][:]
                    )
                nc.sync.dma_start(out=out.ap()[ts(tm, 128), :], in_=stg[:])

        def attention(h):
            kt_sb = p2k.tile([128, nk, T], bf16, tag="kt", name=f"ktsb{h}")
            nc.sync.dma_start(out=kt_sb[:], in_=kt_dram[h][:])
            v_h = p2v.tile([128, TT, 128], bf16, tag="vh", name=f"vh{h}")
            nc.sync.dma_start(out=v_h[:], in_=v_dram[:, :, ts(h, 128)])
            for qc in range(QCH):
                qt_sb = p2q.tile([128, 512], bf16, tag="qt", name=f"qtsb{h}_{qc}")
                nc.sync.dma_start(out=qt_sb[:], in_=qt_dram[h][:, ts(qc, 512)])
                u = p2u.tile(
                    [128, KT // 2, 2, 512], bf16, tag="u", name=f"u{h}_{qc}"
                )
                for kg in range(KT // 2):
                    ps = ps_sc.tile(
                        [128, 2, 512], f32, tag="ps", name=f"sc{h}_{qc}_{kg}"
                    )
                    for kk in range(2):
                        for j in range(nk):
                            nc.tensor.matmul(
                                ps[:, kk, :],
                                lhsT=kt_sb[:, j, ts(2 * kg + kk, 128)],
                                rhs=qt_sb[:],
                                start=(j == 0),
                                stop=(j == nk - 1),
                            )
                    nc.scalar.activation(
                        out=u[:, kg, :, :],
                        in_=ps[:],
                        func=mybir.ActivationFunctionType.Exp,
                        scale=SCALE,
                    )
                # denominator: tree-sum over the 16 k-tiles, then partitions
                s8 = p2sm.tile([128, 8, 512], bf16, tag="s8", name=f"s8_{h}{qc}")
                nc.vector.tensor_add(s8[:], u[:, :, 0, :], u[:, :, 1, :])
                s8v = s8[:].rearrange("p (x y) q -> p x y q", x=4)
                s4 = p2sm.tile([128, 4, 512], bf16, tag="s4", name=f"s4_{h}{qc}")
                nc.vector.tensor_add(s4[:], s8v[:, :, 0, :], s8v[:, :, 1, :])
                s4v = s4[:].rearrange("p (x y) q -> p x y q", x=2)
                s2r = p2sm.tile([128, 3, 512], bf16, tag="s2r", name=f"s2r_{h}{qc}")
                nc.vector.tensor_add(
                    s2r[:, 0:2, :], s4v[:, :, 0, :], s4v[:, :, 1, :]
                )
                nc.vector.tensor_add(s2r[:, 2, :], s2r[:, 0, :], s2r[:, 1, :])
                rsum = p2sm.tile([128, 512], f32, tag="s8", name=f"rs_{h}{qc}")
                nc.gpsimd.partition_all_reduce(
                    rsum[:], s2r[:, 2, :], channels=128,
                    reduce_op=bass_isa.ReduceOp.add
                )
                rrec = p2sm.tile([128, 512], f32, tag="s4", name=f"rr_{h}{qc}")
                nc.vector.reciprocal_approx_fast(out=rrec[:], in_=rsum[:])

                psy = ps_gen.tile([128, 512], f32, tag="ps", name=f"psy{h}_{qc}")
                for kt in range(KT):
                    nc.tensor.matmul(
                        psy[:],
                        lhsT=v_h[:, kt, :],
                        rhs=u[:, kt // 2, kt % 2, :],
                        start=(kt == 0),
                        stop=(kt == KT - 1),
                    )
                nc.vector.tensor_mul(
                    out=yt_sb[:, h, ts(qc, 512)], in0=psy[:], in1=rrec[:]
                )
                if h == HG - 1:
                    oproj_group(qc)

        # ---- phase 1 (+ interleaved attention) ---------------------------
        with (
            tc.tile_pool(name="p1x", bufs=CT) as p1x,
            tc.tile_pool(name="p1wv", bufs=1) as p1wv,
            tc.tile_pool(name="p1cs", bufs=1) as p1cs,
            tc.tile_pool(name="p1w", bufs=1) as p1w,
            tc.tile_pool(name="p1rot", bufs=1) as p1rot,
            tc.tile_pool(name="p1stg", bufs=2) as p1stg,
            tc.tile_pool(name="ps_unused", bufs=1, space="PSUM") as _psu,
        ):
            def load_w(h):
                w_h = {}
                for nm, pack in (("q", wq_pack), ("k", wk_pack)):
                    w = p1w.tile(
                        [128, CT, 128], bf16, tag=f"w{nm}", name=f"w{nm}{h}"
                    )
                    nc.sync.dma_start(out=w[:], in_=pack.ap()[h])
                    w_h[nm] = w
                return w_h

            # head-0 weights and the V weight chunks first so the first
            # matmul chains aren't queued behind the 8MB x^T load
            w_next = load_w(0)
            xts = [
                p1x.tile([128, T], bf16, tag="xt", name=f"xt{ci}")
                for ci in range(CT)
            ]
            # chunk-major loads: the first V/QK accumulation chains only
            # depend on the first t-chunk of every c-tile (subtile deps),
            # cutting the HBM-bound startup stall
            for tc4 in range(QCH):
                for ci in range(CT):
                    nc.sync.dma_start(
                        out=xts[ci][:, ts(tc4, 512)],
                        in_=x_pack.ap()[:, ci, ts(tc4, 512)],
                    )
            cs_sb = p1cs.tile([128, T], f32, tag="cs")
            nc.sync.dma_start(out=cs_sb[:], in_=cs_pack.ap())

            def v_block(dch):
                wv_h = p1wv.tile(
                    [128, CT, 512], bf16, tag="wvh", name=f"wvh{dch}"
                )
                nc.sync.dma_start(
                    out=wv_h[:], in_=wv_pack.ap()[:, :, ts(dch, 512)]
                )
                for tm in range(TT):
                    ps = ps_gen.tile(
                        [128, 512], f32, tag="ps", name=f"vps{dch}_{tm}"
                    )
                    for ci in range(CT):
                        nc.tensor.matmul(
                            ps[:],
                            lhsT=xts[ci][:, ts(tm, 128)],
                            rhs=wv_h[:, ci, :],
                            start=(ci == 0),
                            stop=(ci == CT - 1),
                        )
                    vstg = p1stg.tile(
                        [128, 2, 512], bf16, tag="spl", bufs=4,
                        name=f"vstg{dch}_{tm}"
                    )
                    nc.scalar.copy(out=vstg[:, 0, :], in_=ps[:])
                    nc.sync.dma_start(
                        out=v_dram[:, tm, ts(dch, 512)], in_=vstg[:, 0, :]
                    )

            # V chunk 0 first (heads 0-3), chunk 1 after QK(0): attention(h)
            # only needs its own head's V columns, and the wv DMA stalls are
            # filled by interleaved QK matmuls
            v_block(0)

            # Q^T / K^T per head with attention for head h-1 interleaved
            for h in range(HG):
                w_h = w_next
                for nm in ("q", "k"):
                    for tch in range(QCH):
                        ps = ps_gen.tile(
                            [128, 512], f32, tag="ps", name=f"qk{h}{nm}{tch}"
                        )
                        for ci in range(CT):
                            nc.tensor.matmul(
                                ps[:],
                                lhsT=w_h[nm][:, ci, :],
                                rhs=xts[ci][:, ts(tch, 512)],
                                start=(ci == 0),
                                stop=(ci == CT - 1),
                            )
                        # out1 = x1*cos + x2*sin ; out2 = x1*cos - x2*sin
                        t12 = p1rot.tile([64, 2, 512], f32, tag="t12")
                        t1 = t12[:, 0, :]
                        t2 = t12[:, 1, :]
                        nc.vector.tensor_mul(
                            t1, ps[0:64, :], cs_sb[0:64, ts(tch, 512)]
                        )
                        nc.vector.tensor_mul(
                            t2, ps[64:128, :], cs_sb[64:128, ts(tch, 512)]
                        )
                        if nm == "q" or not SPLIT_K:
                            spill = qt_dram[h] if nm == "q" else kt_dram[h][:, 0, :]
                            stg = p1stg.tile(
                                [128, 2, 512], bf16, tag="spl", bufs=4,
                                name=f"stg{h}{nm}{tch}"
                            )
                            nc.vector.tensor_add(stg[0:64, 0, :], t1, t2)
                            nc.vector.tensor_sub(stg[64:128, 0, :], t1, t2)
                            nc.sync.dma_start(
                                out=spill[:, ts(tch, 512)], in_=stg[:, 0, :]
                            )
                        else:
                            # K split: k = hi + lo (both bf16) removes the
                            # K-side quantization error in the scores matmul
                            stgf = p1stg.tile([128, 512], f32, tag="stgf", bufs=1)
                            nc.vector.tensor_add(stgf[0:64, :], t1, t2)
                            nc.vector.tensor_sub(stgf[64:128, :], t1, t2)
                            khl = p1stg.tile(
                                [128, 2, 512], bf16, tag="spl", bufs=4,
                                name=f"khl{h}{tch}"
                            )
                            nc.vector.tensor_copy(out=khl[:, 0, :], in_=stgf[:])
                            nc.vector.tensor_sub(
                                khl[:, 1, :], stgf[:], khl[:, 0, :]
                            )
                            nc.sync.dma_start(
                                out=kt_dram[h][:, :, ts(tch, 512)], in_=khl[:]
                            )
                if h + 1 < HG:
                    w_next = load_w(h + 1)
                if h == 0:
                    v_block(1)
                if h >= 1:
                    attention(h - 1)

        # o-projection pools: opened after phase 1 releases its SBUF (they
        # alias that zone; the release dep is harmless since o-proj runs
        # at the very end anyway)
        p3wo = ctx.enter_context(tc.tile_pool(name="p3wo", bufs=1))
        p3stg = ctx.enter_context(tc.tile_pool(name="p3stg", bufs=2))
        wo_sb = p3wo.tile([128, HG, C], bf16)
        nc.sync.dma_start(out=wo_sb[:], in_=wo_pack.ap())

        # last head's attention (+ interleaved o-projection groups)
        attention(HG - 1)

    nc.compile()
    return nc


def get_nc():
    if "nc" not in _CACHE:
        _CACHE["nc"] = _build_bass()
    return _CACHE["nc"]


def _pack_inputs(x, cos, sin, wq, wk, wv, wo):
    """Build the 8 per-core input maps (packed, DMA-friendly layouts)."""
    cs = np.concatenate(
        [
            np.asarray(cos[0, :, 0, :], dtype=np.float32).T,  # (64, T)
            np.asarray(sin[0, :, 0, :], dtype=np.float32).T,
        ],
        axis=0,
    )  # (128, T)
    cs = np.ascontiguousarray(cs)
    in_maps = []
    for core in range(N_CORES):
        b, g = divmod(core, 2)
        xb = np.asarray(x[b], dtype=np.float32)  # (T, C)
        # x_pack[ci, co, t] = x[b, t, co*128+ci]
        x_pack = np.ascontiguousarray(
            xb.reshape(T, CT, 128).transpose(2, 1, 0).astype(BF16)
        )
        sl = slice(g * JG, (g + 1) * JG)
        wq_g = np.asarray(wq[sl], dtype=np.float32)  # (JG, C)
        wk_g = np.asarray(wk[sl], dtype=np.float32)
        wv_g = np.asarray(wv[sl], dtype=np.float32)
        wo_g = np.asarray(wo[:, sl], dtype=np.float32)  # (C, JG)
        # wq_pack[h, ci, co, d] = wq_g[h*128+d, co*128+ci]
        wq_pack = np.ascontiguousarray(
            wq_g.reshape(HG, 128, CT, 128).transpose(0, 3, 2, 1).astype(BF16)
        )
        wk_pack = np.ascontiguousarray(
            wk_g.reshape(HG, 128, CT, 128).transpose(0, 3, 2, 1).astype(BF16)
        )
        # wv_pack[ci, co, d] = wv_g[d, co*128+ci]
        wv_pack = np.ascontiguousarray(
            wv_g.reshape(JG, CT, 128).transpose(2, 1, 0).astype(BF16)
        )
        # wo_pack[ji, jo, c] = wo_g[c, jo*128+ji]
        wo_pack = np.ascontiguousarray(
            wo_g.reshape(C, HG, 128).transpose(2, 1, 0).astype(BF16)
        )
        in_maps.append(
            {
                "x_pack": x_pack,
                "wq_pack": wq_pack,
                "wk_pack": wk_pack,
                "wv_pack": wv_pack,
                "wo_pack": wo_pack,
                "cs_pack": cs,
            }
        )
    return in_maps


def run_spmd(in_maps, **kwargs):
    from concourse.bass_utils import run_bass_kernel_spmd

    nc = get_nc()
    return run_bass_kernel_spmd(nc, in_maps, core_ids=list(range(N_CORES)), **kwargs)


def kernel(x, cos, sin, wq, wk, wv, wo):
    in_maps = _pack_inputs(x, cos, sin, wq, wk, wv, wo)
    res = run_spmd(in_maps)
    outs = [r["out"] for r in res.results]
    full = np.empty((B, T, C), dtype=np.float32)
    for b in range(B):
        full[b] = outs[2 * b] + outs[2 * b + 1]
    return full



# revision 2
# speedup vs baseline: 1.1691x; 1.1691x over previous
"""Bidirectional attention (RoPE-variant) Trainium2 kernel.

Reference computation (B=4, T=2048, C=2048, H=16, D=128):
    q = (x @ wq.T) -> rotary; k = (x @ wk.T) -> rotary; v = x @ wv.T
    y = softmax(q k^T / sqrt(D)) v ; out = y @ wo.T

Sharding over 8 NeuronCores: core c -> (batch b = c//2, head-group g = c%2).
Each core computes q/k/v projections for its batch restricted to its 8 heads,
full attention for those heads, and a partial o-projection (contracting its
1024 hidden columns).  The host sums the two partial outputs per batch — no
device collectives, and every core does exactly 1/8 of the matmul FLOPs.

On-device layout is "transposed" end-to-end so the PE (out = lhsT.T @ rhs)
never needs an explicit transpose.  V is computed first (natural (t, d)
layout) and spilled to DRAM; Q^T/K^T are produced head-by-head (rotary
applied straight out of PSUM) and spilled as bf16.  Attention for head h-1
is emitted between the Q/K blocks of head h so its exp() work (ScalarE) and
tree-sums (VectorE) hide under projection matmuls instead of forming an
ACT-bound serial phase.  The softmax denominator's cross-partition reduce is
a single PE matmul against an all-ones stationary operand (213 ns) instead
of a gpsimd partition_all_reduce (~3.5 us) that used to stall the tail.
The partial o-projection is interleaved into the last head's attention loop
and written out in bf16 (host accumulates in f32).
"""

import sys

if "/opt/trn_rl_repo" not in sys.path:
    sys.path.insert(0, "/opt/trn_rl_repo")

import os
import numpy as np
import ml_dtypes

B, T, C = 4, 2048, 2048
H_TOT = 16
D = 128
HG = 8            # heads per core
JG = HG * D       # 1024 hidden columns per head-group
N_CORES = 8
CT = C // 128     # 16 c-tiles (contraction over channels)
TT = T // 128     # 16 t-tiles
QCH = T // 512    # 4 query chunks of 512
KT = T // 128     # 16 key tiles of 128
SCALE = 1.0 / float(np.sqrt(D))

BF16 = ml_dtypes.bfloat16
SPLIT_K = os.environ.get("SPLIT_K", "0") == "1"

_CACHE = {}


def _build_bass():
    import concourse.tile as tile
    from concourse import bacc, mybir
    from concourse.bass import ts
    from contextlib import ExitStack

    bf16 = mybir.dt.bfloat16
    f32 = mybir.dt.float32

    nc = bacc.Bacc("TRN2", target_bir_lowering=False, debug=False)

    x_pack = nc.dram_tensor("x_pack", [128, CT, T], bf16, kind="ExternalInput")
    wq_pack = nc.dram_tensor("wq_pack", [HG, 128, CT, 128], bf16, kind="ExternalInput")
    wk_pack = nc.dram_tensor("wk_pack", [HG, 128, CT, 128], bf16, kind="ExternalInput")
    wv_pack = nc.dram_tensor("wv_pack", [128, CT, JG], bf16, kind="ExternalInput")
    wo_pack = nc.dram_tensor("wo_pack", [128, HG, C], bf16, kind="ExternalInput")
    # cs_pack rows 0:64 = cos^T, rows 64:128 = sin^T
    cs_pack = nc.dram_tensor("cs_pack", [128, T], f32, kind="ExternalInput")
    out = nc.dram_tensor("out", [T, C], bf16, kind="ExternalOutput")

    nk = 2 if SPLIT_K else 1

    with tile.TileContext(nc) as tc, ExitStack() as ctx:
        # Pools opened in lifetime order: persistent + attention scratch first
        # (bottom of the SBUF stack), then phase-1 pools on top, so attention
        # tiles never alias phase-1 space (aliasing would add release deps
        # that serialize the phases).
        persist = ctx.enter_context(tc.tile_pool(name="persist", bufs=1))
        p2k = ctx.enter_context(tc.tile_pool(name="p2k", bufs=2))
        p2q = ctx.enter_context(tc.tile_pool(name="p2q", bufs=2))
        p2u = ctx.enter_context(tc.tile_pool(name="p2u", bufs=2))
        p2sm = ctx.enter_context(tc.tile_pool(name="p2sm", bufs=1))
        p2v = ctx.enter_context(tc.tile_pool(name="p2v", bufs=2))
        dram = ctx.enter_context(tc.tile_pool(name="dram", bufs=1, space="DRAM"))
        ps_sc = ctx.enter_context(tc.tile_pool(name="ps_sc", bufs=2, space="PSUM"))
        ps_gen = ctx.enter_context(tc.tile_pool(name="ps_gen", bufs=3, space="PSUM"))
        ps_red = ctx.enter_context(tc.tile_pool(name="ps_red", bufs=1, space="PSUM"))

        yt_sb = persist.tile([128, HG, T], bf16)      # y^T, (d, h, t)
        ones_sb = persist.tile([128, 128], bf16, tag="ones")
        nc.vector.memset(ones_sb[:], 1.0)

        qt_dram = [
            dram.tile([128, T], bf16, tag=f"qt{h}", name=f"qt_dram{h}")
            for h in range(HG)
        ]
        kt_dram = [
            dram.tile([128, nk, T], bf16, tag=f"kt{h}", name=f"kt_dram{h}")
            for h in range(HG)
        ]
        v_dram = dram.tile([128, TT, JG], bf16, name="v_dram")

        def oproj_group(g):
            # o-proj tile-group g needs q-chunk g of ALL heads, complete at
            # (h = HG-1, qc = g); emitted there so it hides under attention.
            # Two 512-col halves at a time so only 2 PSUM banks are held.
            for tm in range(4 * g, 4 * g + 4):
                stg = p3stg.tile([128, C], bf16, tag="ostg", name=f"ostg{tm}")
                for half in range(2):
                    pss = [
                        ps_gen.tile(
                            [128, 512], f32, tag="ps", name=f"pso{tm}_{half}{c}"
                        )
                        for c in range(2)
                    ]
                    for ji in range(HG):
                        for c2 in range(2):
                            nc.tensor.matmul(
                                pss[c2][:],
                                lhsT=yt_sb[:, ji, ts(tm, 128)],
                                rhs=wo_sb[:, ji, ts(2 * half + c2, 512)],
                                start=(ji == 0),
                                stop=(ji == HG - 1),
                            )
                    for c2 in range(2):
                        nc.vector.tensor_copy(
                            out=stg[:, ts(2 * half + c2, 512)], in_=pss[c2][:]
                        )
                nc.sync.dma_start(out=out.ap()[ts(tm, 128), :], in_=stg[:])

        def load_attention(h):
            kt_sb = p2k.tile([128, nk, T], bf16, tag="kt", name=f"ktsb{h}")
            nc.sync.dma_start(out=kt_sb[:], in_=kt_dram[h][:])
            v_h = p2v.tile([128, TT, 128], bf16, tag="vh", name=f"vh{h}")
            nc.sync.dma_start(out=v_h[:], in_=v_dram[:, :, ts(h, 128)])
            return kt_sb, v_h

        def attention(h, kt_sb, v_h):
            for qc in range(QCH):
                qt_sb = p2q.tile([128, 512], bf16, tag="qt", name=f"qtsb{h}_{qc}")
                nc.sync.dma_start(out=qt_sb[:], in_=qt_dram[h][:, ts(qc, 512)])
                u = p2u.tile(
                    [128, KT // 2, 2, 512], bf16, tag="u", name=f"u{h}_{qc}"
                )
                for kg in range(KT // 2):
                    ps = ps_sc.tile(
                        [128, 2, 512], f32, tag="ps", name=f"sc{h}_{qc}_{kg}"
                    )
                    for kk in range(2):
                        for j in range(nk):
                            nc.tensor.matmul(
                                ps[:, kk, :],
                                lhsT=kt_sb[:, j, ts(2 * kg + kk, 128)],
                                rhs=qt_sb[:],
                                start=(j == 0),
                                stop=(j == nk - 1),
                            )
                    nc.scalar.activation(
                        out=u[:, kg, :, :],
                        in_=ps[:],
                        func=mybir.ActivationFunctionType.Exp,
                        scale=SCALE,
                    )
                # denominator: tree-sum over the 16 k-tiles on VectorE, then
                # a single all-ones matmul folds the 128 partitions (emitted
                # after the attn@V matmuls so the PE never waits on the tree)
                s8 = p2sm.tile([128, 8, 512], bf16, tag="s8", name=f"s8_{h}{qc}")
                nc.vector.tensor_add(s8[:], u[:, :, 0, :], u[:, :, 1, :])
                s8v = s8[:].rearrange("p (x y) q -> p x y q", x=4)
                s4 = p2sm.tile([128, 4, 512], bf16, tag="s4", name=f"s4_{h}{qc}")
                nc.vector.tensor_add(s4[:], s8v[:, :, 0, :], s8v[:, :, 1, :])
                s4v = s4[:].rearrange("p (x y) q -> p x y q", x=2)
                s2r = p2sm.tile([128, 3, 512], bf16, tag="s2r", name=f"s2r_{h}{qc}")
                nc.vector.tensor_add(
                    s2r[:, 0:2, :], s4v[:, :, 0, :], s4v[:, :, 1, :]
                )
                nc.vector.tensor_add(s2r[:, 2, :], s2r[:, 0, :], s2r[:, 1, :])

                psy = ps_gen.tile([128, 512], f32, tag="ps", name=f"psy{h}_{qc}")
                for kt in range(KT):
                    nc.tensor.matmul(
                        psy[:],
                        lhsT=v_h[:, kt, :],
                        rhs=u[:, kt // 2, kt % 2, :],
                        start=(kt == 0),
                        stop=(kt == KT - 1),
                    )
                rsum = ps_red.tile([128, 512], f32, tag="red", name=f"rs_{h}{qc}")
                nc.tensor.matmul(
                    rsum[:], lhsT=ones_sb[:], rhs=s2r[:, 2, :],
                    start=True, stop=True,
                )
                rrec = p2sm.tile([128, 512], f32, tag="s4", name=f"rr_{h}{qc}")
                nc.vector.reciprocal_approx_fast(out=rrec[:], in_=rsum[:])
                nc.vector.tensor_mul(
                    out=yt_sb[:, h, ts(qc, 512)], in0=psy[:], in1=rrec[:]
                )
                if h == HG - 1:
                    oproj_group(qc)

        # ---- phase 1 (+ interleaved attention) ---------------------------
        with (
            tc.tile_pool(name="p1x", bufs=CT) as p1x,
            tc.tile_pool(name="p1wv", bufs=1) as p1wv,
            tc.tile_pool(name="p1cs", bufs=1) as p1cs,
            tc.tile_pool(name="p1w", bufs=1) as p1w,
            tc.tile_pool(name="p1rot", bufs=1) as p1rot,
            tc.tile_pool(name="p1stg", bufs=2) as p1stg,
        ):
            def load_w(h):
                w_h = {}
                for nm, pack in (("q", wq_pack), ("k", wk_pack)):
                    w = p1w.tile(
                        [128, CT, 128], bf16, tag=f"w{nm}", name=f"w{nm}{h}"
                    )
                    nc.sync.dma_start(out=w[:], in_=pack.ap()[h])
                    w_h[nm] = w
                return w_h

            def load_wv(dch):
                wv_h = p1wv.tile(
                    [128, CT, 512], bf16, tag="wvh", name=f"wvh{dch}"
                )
                nc.sync.dma_start(
                    out=wv_h[:], in_=wv_pack.ap()[:, :, ts(dch, 512)]
                )
                return wv_h

            # DMA issue order = DMA queue order: head-0 weights and the
            # first V weight chunk BEFORE the 8MB x^T load so the first
            # matmul chains (V t-tiles 0-3) have all inputs after ~4MB of
            # traffic instead of ~12MB.
            w_next = load_w(0)
            wv_h0 = load_wv(0)
            xts = [
                p1x.tile([128, T], bf16, tag="xt", name=f"xt{ci}")
                for ci in range(CT)
            ]
            # chunk-major loads: the first V/QK accumulation chains only
            # depend on the first t-chunk of every c-tile (subtile deps),
            # cutting the HBM-bound startup stall
            for tc4 in range(QCH):
                for ci in range(CT):
                    nc.sync.dma_start(
                        out=xts[ci][:, ts(tc4, 512)],
                        in_=x_pack.ap()[:, ci, ts(tc4, 512)],
                    )
            # cos/sin only needed at the first rotary (~70us in)
            cs_sb = p1cs.tile([128, T], f32, tag="cs")
            nc.sync.dma_start(out=cs_sb[:], in_=cs_pack.ap())

            def v_block(dch, wv_h):
                for tm in range(TT):
                    ps = ps_gen.tile(
                        [128, 512], f32, tag="ps", name=f"vps{dch}_{tm}"
                    )
                    for ci in range(CT):
                        nc.tensor.matmul(
                            ps[:],
                            lhsT=xts[ci][:, ts(tm, 128)],
                            rhs=wv_h[:, ci, :],
                            start=(ci == 0),
                            stop=(ci == CT - 1),
                        )
                    vstg = p1stg.tile(
                        [128, 2, 512], bf16, tag="spl", bufs=4,
                        name=f"vstg{dch}_{tm}"
                    )
                    nc.scalar.copy(out=vstg[:, 0, :], in_=ps[:])
                    nc.sync.dma_start(
                        out=v_dram[:, tm, ts(dch, 512)], in_=vstg[:, 0, :]
                    )

            # V chunk 0 first (heads 0-3), chunk 1 after QK(0): attention(h)
            # only needs its own head's V columns, and the wv DMA stalls are
            # filled by interleaved QK matmuls
            v_block(0, wv_h0)

            # Q^T / K^T per head with attention for head h-1 interleaved
            att_tiles = None
            for h in range(HG):
                w_h = w_next
                if h >= 1:
                    # issue the kt/v reloads for attention(h-1) ahead of the
                    # QK(h) spill writes so they aren't queued behind them
                    att_tiles = load_attention(h - 1)
                for nm in ("q", "k"):
                    for tch in range(QCH):
                        ps = ps_gen.tile(
                            [128, 512], f32, tag="ps", name=f"qk{h}{nm}{tch}"
                        )
                        for ci in range(CT):
                            nc.tensor.matmul(
                                ps[:],
                                lhsT=w_h[nm][:, ci, :],
                                rhs=xts[ci][:, ts(tch, 512)],
                                start=(ci == 0),
                                stop=(ci == CT - 1),
                            )
                        # out1 = x1*cos + x2*sin ; out2 = x1*cos - x2*sin
                        t12 = p1rot.tile([64, 2, 512], f32, tag="t12")
                        t1 = t12[:, 0, :]
                        t2 = t12[:, 1, :]
                        nc.vector.tensor_mul(
                            t1, ps[0:64, :], cs_sb[0:64, ts(tch, 512)]
                        )
                        nc.vector.tensor_mul(
                            t2, ps[64:128, :], cs_sb[64:128, ts(tch, 512)]
                        )
                        if nm == "q" or not SPLIT_K:
                            spill = qt_dram[h] if nm == "q" else kt_dram[h][:, 0, :]
                            stg = p1stg.tile(
                                [128, 2, 512], bf16, tag="spl", bufs=4,
                                name=f"stg{h}{nm}{tch}"
                            )
                            nc.vector.tensor_add(stg[0:64, 0, :], t1, t2)
                            nc.vector.tensor_sub(stg[64:128, 0, :], t1, t2)
                            nc.sync.dma_start(
                                out=spill[:, ts(tch, 512)], in_=stg[:, 0, :]
                            )
                        else:
                            # K split: k = hi + lo (both bf16) removes the
                            # K-side quantization error in the scores matmul
                            stgf = p1stg.tile([128, 512], f32, tag="stgf", bufs=1)
                            nc.vector.tensor_add(stgf[0:64, :], t1, t2)
                            nc.vector.tensor_sub(stgf[64:128, :], t1, t2)
                            khl = p1stg.tile(
                                [128, 2, 512], bf16, tag="spl", bufs=4,
                                name=f"khl{h}{tch}"
                            )
                            nc.vector.tensor_copy(out=khl[:, 0, :], in_=stgf[:])
                            nc.vector.tensor_sub(
                                khl[:, 1, :], stgf[:], khl[:, 0, :]
                            )
                            nc.sync.dma_start(
                                out=kt_dram[h][:, :, ts(tch, 512)], in_=khl[:]
                            )
                if h + 1 < HG:
                    w_next = load_w(h + 1)
                if h == 0:
                    v_block(1, load_wv(1))
                if h >= 1:
                    attention(h - 1, *att_tiles)

        # o-projection pools: opened after phase 1 releases its SBUF (they
        # alias that zone; the release dep is harmless since o-proj runs
        # at the very end anyway)
        p3wo = ctx.enter_context(tc.tile_pool(name="p3wo", bufs=1))
        p3stg = ctx.enter_context(tc.tile_pool(name="p3stg", bufs=2))

        # attention(7)'s kt/v loads go on the DMA queue before the 4MB wo
        # load; wo itself is split per-ji so the first o-proj accumulation
        # (ji=0) can start after 0.5MB (subtile deps)
        att7 = load_attention(HG - 1)
        wo_sb = p3wo.tile([128, HG, C], bf16)
        for ji in range(HG):
            nc.sync.dma_start(out=wo_sb[:, ji, :], in_=wo_pack.ap()[:, ji, :])

        # last head's attention (+ interleaved o-projection groups)
        attention(HG - 1, *att7)

    nc.compile()
    return nc


def get_nc():
    if "nc" not in _CACHE:
        _CACHE["nc"] = _build_bass()
    return _CACHE["nc"]


def _pack_inputs(x, cos, sin, wq, wk, wv, wo):
    """Build the 8 per-core input maps (packed, DMA-friendly layouts)."""
    cs = np.concatenate(
        [
            np.asarray(cos[0, :, 0, :], dtype=np.float32).T,  # (64, T)
            np.asarray(sin[0, :, 0, :], dtype=np.float32).T,
        ],
        axis=0,
    )  # (128, T)
    cs = np.ascontiguousarray(cs)
    in_maps = []
    for core in range(N_CORES):
        b, g = divmod(core, 2)
        xb = np.asarray(x[b], dtype=np.float32)  # (T, C)
        # x_pack[ci, co, t] = x[b, t, co*128+ci]
        x_pack = np.ascontiguousarray(
            xb.reshape(T, CT, 128).transpose(2, 1, 0).astype(BF16)
        )
        sl = slice(g * JG, (g + 1) * JG)
        wq_g = np.asarray(wq[sl], dtype=np.float32)  # (JG, C)
        wk_g = np.asarray(wk[sl], dtype=np.float32)
        wv_g = np.asarray(wv[sl], dtype=np.float32)
        wo_g = np.asarray(wo[:, sl], dtype=np.float32)  # (C, JG)
        # wq_pack[h, ci, co, d] = wq_g[h*128+d, co*128+ci]
        wq_pack = np.ascontiguousarray(
            wq_g.reshape(HG, 128, CT, 128).transpose(0, 3, 2, 1).astype(BF16)
        )
        wk_pack = np.ascontiguousarray(
            wk_g.reshape(HG, 128, CT, 128).transpose(0, 3, 2, 1).astype(BF16)
        )
        # wv_pack[ci, co, d] = wv_g[d, co*128+ci]
        wv_pack = np.ascontiguousarray(
            wv_g.reshape(JG, CT, 128).transpose(2, 1, 0).astype(BF16)
        )
        # wo_pack[ji, jo, c] = wo_g[c, jo*128+ji]
        wo_pack = np.ascontiguousarray(
            wo_g.reshape(C, HG, 128).transpose(2, 1, 0).astype(BF16)
        )
        in_maps.append(
            {
                "x_pack": x_pack,
                "wq_pack": wq_pack,
                "wk_pack": wk_pack,
                "wv_pack": wv_pack,
                "wo_pack": wo_pack,
                "cs_pack": cs,
            }
        )
    return in_maps


def run_spmd(in_maps, **kwargs):
    from concourse.bass_utils import run_bass_kernel_spmd

    nc = get_nc()
    return run_bass_kernel_spmd(nc, in_maps, core_ids=list(range(N_CORES)), **kwargs)


def kernel(x, cos, sin, wq, wk, wv, wo):
    in_maps = _pack_inputs(x, cos, sin, wq, wk, wv, wo)
    res = run_spmd(in_maps)
    outs = [np.asarray(r["out"], dtype=np.float32) for r in res.results]
    full = np.empty((B, T, C), dtype=np.float32)
    for b in range(B):
        full[b] = outs[2 * b] + outs[2 * b + 1]
    return full


# revision 8
# speedup vs baseline: 1.2058x; 1.0314x over previous
"""Bidirectional attention (RoPE-variant) Trainium2 kernel.

Reference computation (B=4, T=2048, C=2048, H=16, D=128):
    q = (x @ wq.T) -> rotary; k = (x @ wk.T) -> rotary; v = x @ wv.T
    y = softmax(q k^T / sqrt(D)) v ; out = y @ wo.T

Sharding over 8 NeuronCores: core c -> (batch b = c//2, head-group g = c%2).
Each core computes q/k/v projections for its batch restricted to its 8 heads,
full attention for those heads, and a partial o-projection (contracting its
1024 hidden columns).  The host sums the two partial outputs per batch — no
device collectives, and every core does exactly 1/8 of the matmul FLOPs.

Schedule: V is produced first (two wv quarters), then per head-window h the
Q/K projection chains for head h are emitted with attention micro-blocks for
head h-1 interleaved between them: SC(qc) = scores+exp for one 512-query
chunk, AV(qc) = tree-sum + attn@V + denominator-reduce + normalize.  The PE
queue is in-order, so this interleave is what lets the ScalarE exp() time
(~38us/window) hide under projection matmuls; AV(qc) is placed ~3 chains
after SC(qc) so exp has drained by then.  K^T stays resident in SBUF in f32
(rotary writes it directly; no spill), Q^T spills to DRAM in f32, and the
scores matmul consumes both as float32r — same PE throughput as bf16 at
N=512 (~227ns vs 216ns measured) with ~18x better precision, eliminating
the q/k quantization error that dominates exp(scores).  The softmax
denominator's cross-partition reduce is a single PE matmul against an
all-ones stationary operand.  The partial o-projection interleaves into the
last head's attention and is written out in bf16 (host accumulates in f32).
"""

import sys

if "/opt/trn_rl_repo" not in sys.path:
    sys.path.insert(0, "/opt/trn_rl_repo")

import numpy as np
import ml_dtypes

B, T, C = 4, 2048, 2048
H_TOT = 16
D = 128
HG = 8            # heads per core
JG = HG * D       # 1024 hidden columns per head-group
N_CORES = 8
CT = C // 128     # 16 c-tiles (contraction over channels)
TT = T // 128     # 16 t-tiles
QCH = T // 512    # 4 query chunks of 512
KT = T // 128     # 16 key tiles of 128
VQ = JG // 256    # 4 wv quarters
SCALE = 1.0 / float(np.sqrt(D))

BF16 = ml_dtypes.bfloat16

_CACHE = {}


def _build_bass():
    import concourse.tile as tile
    from concourse import bacc, mybir
    from concourse.bass import ts
    from contextlib import ExitStack

    bf16 = mybir.dt.bfloat16
    f32 = mybir.dt.float32
    f32r = mybir.dt.float32r

    nc = bacc.Bacc("TRN2", target_bir_lowering=False, debug=False)

    # x/wv are packed chunk-major so each load is one DMA with fat
    # per-partition-contiguous descriptors on both sides — startup is
    # DMA-bound, so descriptor efficiency sets the PE start time.
    x_pack = nc.dram_tensor("x_pack", [QCH, 128, CT, 512], bf16, kind="ExternalInput")
    wq_pack = nc.dram_tensor("wq_pack", [HG, 128, CT, 128], bf16, kind="ExternalInput")
    wk_pack = nc.dram_tensor("wk_pack", [HG, 128, CT, 128], bf16, kind="ExternalInput")
    wv_pack = nc.dram_tensor("wv_pack", [VQ, 128, CT, 256], bf16, kind="ExternalInput")
    wo_pack = nc.dram_tensor("wo_pack", [128, HG, C], bf16, kind="ExternalInput")
    # cs_pack rows 0:64 = cos^T, rows 64:128 = sin^T
    cs_pack = nc.dram_tensor("cs_pack", [128, T], f32, kind="ExternalInput")
    out = nc.dram_tensor("out", [T, C], bf16, kind="ExternalOutput")

    with tile.TileContext(nc) as tc, ExitStack() as ctx:
        # Pools opened in lifetime order: persistent + attention scratch first
        # (bottom of the SBUF stack), then phase-1 pools on top, so attention
        # tiles never alias phase-1 space.
        persist = ctx.enter_context(tc.tile_pool(name="persist", bufs=1))
        p2k = ctx.enter_context(tc.tile_pool(name="p2k", bufs=2))
        p2q = ctx.enter_context(tc.tile_pool(name="p2q", bufs=2))
        p2u = ctx.enter_context(tc.tile_pool(name="p2u", bufs=2))
        p2sm = ctx.enter_context(tc.tile_pool(name="p2sm", bufs=1))
        p2v = ctx.enter_context(tc.tile_pool(name="p2v", bufs=2))
        dram = ctx.enter_context(tc.tile_pool(name="dram", bufs=1, space="DRAM"))
        ps_sc = ctx.enter_context(tc.tile_pool(name="ps_sc", bufs=2, space="PSUM"))
        ps_gen = ctx.enter_context(tc.tile_pool(name="ps_gen", bufs=3, space="PSUM"))
        ps_red = ctx.enter_context(tc.tile_pool(name="ps_red", bufs=1, space="PSUM"))

        yt_sb = persist.tile([128, HG, T], bf16)      # y^T, (d, h, t)
        ones_sb = persist.tile([128, 128], bf16, tag="ones")
        nc.vector.memset(ones_sb[:], 1.0)

        qt_dram = [
            dram.tile([128, T], f32r, tag=f"qt{h}", name=f"qt_dram{h}")
            for h in range(HG)
        ]
        v_dram = dram.tile([128, TT, JG], bf16, name="v_dram")

        # ---- attention micro-block emitters (head h, interleaved) --------
        class Att:
            def __init__(self, h, kt_sb):
                self.h = h
                self.kt = kt_sb
                v_h = p2v.tile([128, TT, 128], bf16, tag="vh", name=f"vh{h}")
                nc.sync.dma_start(out=v_h[:], in_=v_dram[:, :, ts(h, 128)])
                self.v = v_h
                self.qts = {}
                self.us = {}
                self.prefetch_qt(0)

            def prefetch_qt(self, qc):
                if qc >= QCH or qc in self.qts:
                    return
                qt = p2q.tile(
                    [128, 512], f32r, tag="qt", name=f"qt{self.h}_{qc}"
                )
                nc.sync.dma_start(
                    out=qt[:], in_=qt_dram[self.h][:, ts(qc, 512)]
                )
                self.qts[qc] = qt

            def sc(self, qc):
                h = self.h
                self.prefetch_qt(qc + 1)
                u = p2u.tile(
                    [128, KT // 2, 2, 512], bf16, tag="u", name=f"u{h}_{qc}"
                )
                self.us[qc] = u
                qt_r = self.qts[qc][:]
                for kg in range(KT // 2):
                    ps = ps_sc.tile(
                        [128, 2, 512], f32, tag="ps", name=f"sc{h}_{qc}_{kg}"
                    )
                    for kk in range(2):
                        nc.tensor.matmul(
                            ps[:, kk, :],
                            lhsT=self.kt[:, ts(2 * kg + kk, 128)],
                            rhs=qt_r,
                            start=True,
                            stop=True,
                        )
                    nc.scalar.activation(
                        out=u[:, kg, :, :],
                        in_=ps[:],
                        func=mybir.ActivationFunctionType.Exp,
                        scale=SCALE,
                    )

            def av(self, qc):
                h = self.h
                u = self.us.pop(qc)
                self.qts.pop(qc)
                # denominator tree-sum on VectorE; exp(qc) has drained by the
                # time this block is emitted, so these don't head-block the
                # (in-order) Vector queue
                s8 = p2sm.tile([128, 8, 512], bf16, tag="s8", name=f"s8_{h}{qc}")
                nc.vector.tensor_add(s8[:], u[:, :, 0, :], u[:, :, 1, :])
                s8v = s8[:].rearrange("p (x y) q -> p x y q", x=4)
                s4 = p2sm.tile([128, 4, 512], bf16, tag="s4", name=f"s4_{h}{qc}")
                nc.vector.tensor_add(s4[:], s8v[:, :, 0, :], s8v[:, :, 1, :])
                s4v = s4[:].rearrange("p (x y) q -> p x y q", x=2)
                s2r = p2sm.tile([128, 3, 512], bf16, tag="s2r", name=f"s2r_{h}{qc}")
                nc.vector.tensor_add(
                    s2r[:, 0:2, :], s4v[:, :, 0, :], s4v[:, :, 1, :]
                )
                nc.vector.tensor_add(s2r[:, 2, :], s2r[:, 0, :], s2r[:, 1, :])

                psy = ps_gen.tile([128, 512], f32, tag="ps", name=f"psy{h}_{qc}")
                for kt in range(KT):
                    nc.tensor.matmul(
                        psy[:],
                        lhsT=self.v[:, kt, :],
                        rhs=u[:, kt // 2, kt % 2, :],
                        start=(kt == 0),
                        stop=(kt == KT - 1),
                    )
                # cross-partition reduce of the denominator: one all-ones
                # matmul (~0.2us) instead of a ~3.5us gpsimd all-reduce
                rsum = ps_red.tile([128, 512], f32, tag="red", name=f"rs_{h}{qc}")
                nc.tensor.matmul(
                    rsum[:], lhsT=ones_sb[:], rhs=s2r[:, 2, :],
                    start=True, stop=True,
                )
                rrec = p2sm.tile([128, 512], f32, tag="s4", name=f"rr_{h}{qc}")
                nc.vector.reciprocal_approx_fast(out=rrec[:], in_=rsum[:])
                nc.vector.tensor_mul(
                    out=yt_sb[:, h, ts(qc, 512)], in0=psy[:], in1=rrec[:]
                )

        def oproj_group(g):
            # o-proj tile-group g needs q-chunk g of ALL heads; emitted in
            # the tail right after yt(7, g) is finalized.  Two 512-col
            # halves at a time so only 2 PSUM banks are held; each half's
            # output is DMA'd as soon as its copies finish.
            for tm in range(4 * g, 4 * g + 4):
                stg = p3stg.tile([128, C], bf16, tag="ostg", name=f"ostg{tm}")
                for half in range(2):
                    pss = [
                        ps_gen.tile(
                            [128, 512], f32, tag="ps", name=f"pso{tm}_{half}{c}"
                        )
                        for c in range(2)
                    ]
                    for ji in range(HG):
                        for c2 in range(2):
                            nc.tensor.matmul(
                                pss[c2][:],
                                lhsT=yt_sb[:, ji, ts(tm, 128)],
                                rhs=wo_sb[:, ji, ts(2 * half + c2, 512)],
                                start=(ji == 0),
                                stop=(ji == HG - 1),
                            )
                    for c2 in range(2):
                        nc.vector.tensor_copy(
                            out=stg[:, ts(2 * half + c2, 512)], in_=pss[c2][:]
                        )
                    nc.sync.dma_start(
                        out=out.ap()[ts(tm, 128), ts(half, 1024)],
                        in_=stg[:, ts(half, 1024)],
                    )

        # ---- phase 1: projections with interleaved attention -------------
        with (
            tc.tile_pool(name="p1x", bufs=1) as p1x,
            tc.tile_pool(name="p1wv", bufs=1) as p1wv,
            tc.tile_pool(name="p1cs", bufs=1) as p1cs,
            tc.tile_pool(name="p1w", bufs=1) as p1w,
            tc.tile_pool(name="p1rot", bufs=1) as p1rot,
            tc.tile_pool(name="p1stg", bufs=2) as p1stg,
        ):
            def load_w(h):
                w_h = {}
                for nm, pack in (("q", wq_pack), ("k", wk_pack)):
                    w = p1w.tile(
                        [128, CT, 128], bf16, tag=f"w{nm}", name=f"w{nm}{h}"
                    )
                    nc.sync.dma_start(out=w[:], in_=pack.ap()[h])
                    w_h[nm] = w
                return w_h

            def load_wv(qd):
                wv_h = p1wv.tile(
                    [128, CT, 256], bf16, tag="wvh", name=f"wvh{qd}"
                )
                nc.sync.dma_start(out=wv_h[:], in_=wv_pack.ap()[qd])
                return wv_h

            # DMA issue order = DMA queue order: head-0 weights and the
            # first V weight quarter BEFORE the 8MB x^T load so the first
            # matmul chains have their inputs after ~3.5MB of traffic.
            w_next = load_w(0)
            wv_h0 = load_wv(0)
            # x^T in SBUF, chunk-major: x_sb[:, tc4, ci, t'] — each chunk is
            # one fully-contiguous DMA; the first V/QK chains only depend on
            # chunk 0 (subtile deps).
            x_sb = p1x.tile([128, QCH, CT, 512], bf16, tag="xt")
            for tc4 in range(QCH):
                nc.sync.dma_start(out=x_sb[:, tc4, :, :], in_=x_pack.ap()[tc4])
            # cos/sin only needed at the first rotary (~60us in)
            cs_sb = p1cs.tile([128, T], f32, tag="cs")
            nc.sync.dma_start(out=cs_sb[:], in_=cs_pack.ap())

            def v_quarter(qd, wv_h):
                # V columns [qd*256, qd*256+256) for all T (heads 2qd, 2qd+1)
                for tm in range(TT):
                    ps = ps_gen.tile(
                        [128, 256], f32, tag="ps", name=f"vps{qd}_{tm}"
                    )
                    for ci in range(CT):
                        nc.tensor.matmul(
                            ps[:],
                            lhsT=x_sb[:, tm // 4, ci, ts(tm % 4, 128)],
                            rhs=wv_h[:, ci, :],
                            start=(ci == 0),
                            stop=(ci == CT - 1),
                        )
                    vstg = p1stg.tile(
                        [128, 256], bf16, tag="vst", bufs=2,
                        name=f"vstg{qd}_{tm}"
                    )
                    nc.scalar.copy(out=vstg[:], in_=ps[:])
                    nc.sync.dma_start(
                        out=v_dram[:, tm, ts(qd, 256)], in_=vstg[:]
                    )

            def qk_chain(h, w_h, kt_cur, nm, tch):
                ps = ps_gen.tile(
                    [128, 512], f32, tag="ps", name=f"qk{h}{nm}{tch}"
                )
                for ci in range(CT):
                    nc.tensor.matmul(
                        ps[:],
                        lhsT=w_h[nm][:, ci, :],
                        rhs=x_sb[:, tch, ci, :],
                        start=(ci == 0),
                        stop=(ci == CT - 1),
                    )
                # rotary: out1 = x1*cos + x2*sin ; out2 = x1*cos - x2*sin
                t12 = p1rot.tile([64, 2, 512], f32, tag="t12")
                t1 = t12[:, 0, :]
                t2 = t12[:, 1, :]
                nc.vector.tensor_mul(t1, ps[0:64, :], cs_sb[0:64, ts(tch, 512)])
                nc.vector.tensor_mul(
                    t2, ps[64:128, :], cs_sb[64:128, ts(tch, 512)]
                )
                if nm == "k":
                    # K^T written straight into its resident SBUF tile
                    nc.vector.tensor_add(kt_cur[0:64, ts(tch, 512)], t1, t2)
                    nc.vector.tensor_sub(kt_cur[64:128, ts(tch, 512)], t1, t2)
                else:
                    # Q^T spilled to DRAM in f32 (consumed as float32r)
                    stg = p1stg.tile(
                        [128, 512], f32r, tag="spl", bufs=2,
                        name=f"stg{h}{nm}{tch}"
                    )
                    nc.vector.tensor_add(stg[0:64, :], t1, t2)
                    nc.vector.tensor_sub(stg[64:128, :], t1, t2)
                    nc.sync.dma_start(
                        out=qt_dram[h][:, ts(tch, 512)], in_=stg[:]
                    )

            # V quarters 0,1 (heads 0-3) up front; 2,3 inside window 0
            v_quarter(0, wv_h0)
            v_quarter(1, load_wv(1))

            atts = {}
            kts = {}
            for h in range(HG):
                w_h = w_next
                kt_cur = p2k.tile([128, T], f32r, tag="kt", name=f"kt{h}")
                kts[h] = kt_cur
                if h >= 1:
                    atts[h - 1] = Att(h - 1, kts[h - 1])
                # interleave schedule: after QK chain i of window h, emit
                # attention micro-block inserts[i].  AV(qc) trails SC(qc) by
                # >=3 chains (~15us) so the ScalarE exp has drained; the
                # last two AV blocks of head h-1 ride in window h+1.
                a_prev = atts.get(h - 2)   # AV(h-2, 2/3) pending
                a_cur = atts.get(h - 1)
                inserts = [
                    (a_prev, "av", 2),
                    (a_prev, "av", 3),
                    (a_cur, "sc", 0),
                    (a_cur, "sc", 1),
                    (a_cur, "av", 0),
                    (a_cur, "sc", 2),
                    (a_cur, "av", 1),
                    (a_cur, "sc", 3),
                ]
                chains = [(nm, tch) for nm in ("q", "k") for tch in range(QCH)]
                for i, (nm, tch) in enumerate(chains):
                    qk_chain(h, w_h, kt_cur, nm, tch)
                    a, kind, qc = inserts[i]
                    if a is not None:
                        getattr(a, kind)(qc)
                    if h == 0 and nm == "q" and tch == 3:
                        v_quarter(2, load_wv(2))
                    if h == 0 and nm == "k" and tch == 3:
                        v_quarter(3, load_wv(3))
                if h + 1 < HG:
                    w_next = load_w(h + 1)
                if h - 2 in atts:
                    del atts[h - 2]

        # ---- tail: attention(7) + pending AV(6) + o-projection -----------
        p3wo = ctx.enter_context(tc.tile_pool(name="p3wo", bufs=1))
        p3stg = ctx.enter_context(tc.tile_pool(name="p3stg", bufs=2))

        a6 = atts[HG - 2]
        a7 = Att(HG - 1, kts[HG - 1])
        # only qt(0)/qt(1) fit the 2-buf ring up front; a third prefetch's
        # DMA would wait on a buffer release at the HEAD of the in-order DMA
        # queue and block the wo loads behind it
        a7.prefetch_qt(1)
        wo_sb = p3wo.tile([128, HG, C], bf16)
        for ji in range(HG):
            nc.sync.dma_start(out=wo_sb[:, ji, :], in_=wo_pack.ap()[:, ji, :])

        a6.av(2)
        a7.sc(0)
        a6.av(3)
        a7.sc(1)
        a7.av(0)
        oproj_group(0)
        a7.sc(2)
        a7.av(1)
        oproj_group(1)
        a7.sc(3)
        a7.av(2)
        oproj_group(2)
        a7.av(3)
        oproj_group(3)

    nc.compile()
    return nc


def get_nc():
    if "nc" not in _CACHE:
        _CACHE["nc"] = _build_bass()
    return _CACHE["nc"]


def _pack_inputs(x, cos, sin, wq, wk, wv, wo):
    """Build the 8 per-core input maps (packed, DMA-friendly layouts)."""
    cs = np.concatenate(
        [
            np.asarray(cos[0, :, 0, :], dtype=np.float32).T,  # (64, T)
            np.asarray(sin[0, :, 0, :], dtype=np.float32).T,
        ],
        axis=0,
    )  # (128, T)
    cs = np.ascontiguousarray(cs)
    in_maps = []
    for core in range(N_CORES):
        b, g = divmod(core, 2)
        xb = np.asarray(x[b], dtype=np.float32)  # (T, C)
        # x_pack[tc4, p, ci, t'] = x[b, tc4*512+t', ci*128+p]
        x_pack = np.ascontiguousarray(
            xb.reshape(QCH, 512, CT, 128).transpose(0, 3, 2, 1).astype(BF16)
        )
        sl = slice(g * JG, (g + 1) * JG)
        wq_g = np.asarray(wq[sl], dtype=np.float32)  # (JG, C)
        wk_g = np.asarray(wk[sl], dtype=np.float32)
        wv_g = np.asarray(wv[sl], dtype=np.float32)
        wo_g = np.asarray(wo[:, sl], dtype=np.float32)  # (C, JG)
        # wq_pack[h, ci, co, d] = wq_g[h*128+d, co*128+ci]
        wq_pack = np.ascontiguousarray(
            wq_g.reshape(HG, 128, CT, 128).transpose(0, 3, 2, 1).astype(BF16)
        )
        wk_pack = np.ascontiguousarray(
            wk_g.reshape(HG, 128, CT, 128).transpose(0, 3, 2, 1).astype(BF16)
        )
        # wv_pack[qd, ci, co, d'] = wv_g[qd*256+d', co*128+ci]
        wv_pack = np.ascontiguousarray(
            wv_g.reshape(VQ, 256, CT, 128).transpose(0, 3, 2, 1).astype(BF16)
        )
        # wo_pack[ji, jo, c] = wo_g[c, jo*128+ji]
        wo_pack = np.ascontiguousarray(
            wo_g.reshape(C, HG, 128).transpose(2, 1, 0).astype(BF16)
        )
        in_maps.append(
            {
                "x_pack": x_pack,
                "wq_pack": wq_pack,
                "wk_pack": wk_pack,
                "wv_pack": wv_pack,
                "wo_pack": wo_pack,
                "cs_pack": cs,
            }
        )
    return in_maps


def run_spmd(in_maps, **kwargs):
    from concourse.bass_utils import run_bass_kernel_spmd

    nc = get_nc()
    return run_bass_kernel_spmd(nc, in_maps, core_ids=list(range(N_CORES)), **kwargs)


def kernel(x, cos, sin, wq, wk, wv, wo):
    in_maps = _pack_inputs(x, cos, sin, wq, wk, wv, wo)
    res = run_spmd(in_maps)
    outs = [np.asarray(r["out"], dtype=np.float32) for r in res.results]
    full = np.empty((B, T, C), dtype=np.float32)
    for b in range(B):
        full[b] = outs[2 * b] + outs[2 * b + 1]
    return full
